# revision 4
# baseline (speedup 1.0000x reference)
"""GRU encoder (nn_Encoder) Trainium2 Bass kernel.

Model: emb = E[x]; xin = emb @ W + b[0]; then T=128 GRU steps (reset_after):
    rec = h @ U + b[1]
    z = sigmoid(xin_z + rec_z); r = sigmoid(xin_r + rec_r)
    hh = tanh(xin_h + r * rec_h)
    h  = z*h + (1-z)*hh

Sharding: data-parallel over batch, 8 rows per core. All weights replicated
(bf16); state/gates kept transposed (units on partitions, batch on free dim)
so the recurrent matmul out^T = U^T @ h^T lands directly in gate layout.

Per-core layouts:
  hT   [128, 64]      col kk*8+b  = h[b, kk*128+p]          (kk = unit block)
  xinT [128, 24, BT]  [p, m, t*8+b] = xin[b, t, m*128+p]
  yT   [128, T*64]    col t*64+kk*8+b = h_t[b, kk*128+p]
"""

import numpy as np
import ml_dtypes

import concourse.bass as bass
import concourse.mybir as mybir
from concourse import bacc
from concourse.tile import TileContext
from concourse import bass_utils
from concourse.masks import make_identity

BF = ml_dtypes.bfloat16
VOCAB, EMB, UNITS, B, T = 32000, 256, 1024, 64, 128
NCORES = 8
BLOC = B // NCORES          # 8 batch rows per core
BT = BLOC * T               # 1024 gathered rows per core
KU = UNITS // 128           # 8 K tiles over hidden units
MU = 3 * UNITS // 128       # 24 M tiles over rec columns
GW = BLOC * KU              # 64: gate tile free width (8 unit-blocks x 8 batch)

f32 = mybir.dt.float32
bf16 = mybir.dt.bfloat16
i32 = mybir.dt.int32

LAST_RESULTS = {}           # test harness peeks here for exec_time_ns


def build_nc(t_steps=T):
    bt = BLOC * t_steps
    nch = min(512, bt)
    nc = bacc.Bacc("TRN2")
    Ein = nc.dram_tensor("E", [VOCAB, EMB], bf16, kind="ExternalInput")
    Win = nc.dram_tensor("W", [EMB, 3 * UNITS], bf16, kind="ExternalInput")
    Uin = nc.dram_tensor("U", [UNITS, 3 * UNITS], bf16, kind="ExternalInput")
    idx_in = nc.dram_tensor("idx", [128, bt // 128], i32, kind="ExternalInput")
    hT0_in = nc.dram_tensor("hT0", [128, GW], f32, kind="ExternalInput")
    bias0_in = nc.dram_tensor("bias0", [128, MU], f32, kind="ExternalInput")
    b1h_in = nc.dram_tensor("b1h", [128, GW], f32, kind="ExternalInput")
    yT = nc.dram_tensor("yT", [128, t_steps * GW], f32, kind="ExternalOutput")

    with TileContext(nc) as tc:
        with (
            tc.tile_pool(name="const", bufs=1) as cpool,
            tc.tile_pool(name="work", bufs=2) as wpool,
        ):
            ident = cpool.tile([128, 128], bf16)
            make_identity(nc, ident[:])
            idx_sb = cpool.tile([128, bt // 128], i32)
            nc.sync.dma_start(idx_sb[:], idx_in[:])
            sb_W = cpool.tile([128, EMB // 128, 3 * UNITS], bf16)
            nc.sync.dma_start(
                sb_W[:], Win[:].rearrange("(kk p) m -> p kk m", p=128)
            )
            sb_U = cpool.tile([128, KU, 3 * UNITS], bf16)
            nc.sync.dma_start(
                sb_U[:], Uin[:].rearrange("(kk p) m -> p kk m", p=128)
            )
            sb_bias0 = cpool.tile([128, MU], f32)
            nc.sync.dma_start(sb_bias0[:], bias0_in[:])
            sb_b1h = cpool.tile([128, GW], f32)
            nc.sync.dma_start(sb_b1h[:], b1h_in[:])

            hTf = cpool.tile([128, GW], f32, tag="hTf0")
            nc.sync.dma_start(hTf[:], hT0_in[:])
            hTb = cpool.tile([128, GW], bf16, tag="hTb0")
            nc.vector.tensor_copy(hTb[:], hTf[:])

            # Phase 1: embedding gather + transpose -> embT [128, 2, BT] bf16
            embT = cpool.tile([128, EMB // 128, bt], bf16)
            with tc.tile_pool(name="ptr", bufs=2, space="PSUM") as ptr:
                for tt in range(bt // 128):
                    etile = wpool.tile([128, EMB], bf16, tag="etile")
                    nc.gpsimd.indirect_dma_start(
                        out=etile[:], out_offset=None, in_=Ein[:],
                        in_offset=bass.IndirectOffsetOnAxis(
                            ap=idx_sb[:, tt:tt + 1], axis=0),
                    )
                    for ee in range(EMB // 128):
                        pt = ptr.tile([128, 128], bf16)
                        nc.tensor.transpose(
                            out=pt[:], in_=etile[:, ee * 128:(ee + 1) * 128],
                            identity=ident[:])
                        nc.vector.tensor_copy(
                            out=embT[:, ee, tt * 128:(tt + 1) * 128], in_=pt[:])

            # Phase 2: input GEMM -> xinT [128, MU, BT] f32 (bias folded in)
            xinT = cpool.tile([128, MU, bt], f32)
            with tc.tile_pool(name="px", bufs=4, space="PSUM") as pxp:
                for mm in range(MU):
                    for nh in range(bt // nch):
                        px = pxp.tile([128, nch], f32)
                        for kk in range(EMB // 128):
                            nc.tensor.matmul(
                                px[:],
                                sb_W[:, kk, mm * 128:(mm + 1) * 128],
                                embT[:, kk, nh * nch:(nh + 1) * nch],
                                start=(kk == 0), stop=(kk == EMB // 128 - 1),
                            )
                        nc.scalar.activation(
                            out=xinT[:, mm, nh * nch:(nh + 1) * nch], in_=px[:],
                            func=mybir.ActivationFunctionType.Identity,
                            bias=sb_bias0[:, mm:mm + 1],
                        )

            # Phase 3: recurrence
            with tc.tile_pool(name="pg", bufs=2, space="PSUM") as pgp:
                for t in range(t_steps):
                    pz = pgp.tile([128, GW], f32, tag="pz")
                    pr = pgp.tile([128, GW], f32, tag="pr")
                    ph = pgp.tile([128, GW], f32, tag="ph")
                    for g, pg in ((0, pz), (1, pr), (2, ph)):
                        for mm in range(KU):
                            o = pg[:, mm * BLOC:(mm + 1) * BLOC]
                            for kk in range(KU):
                                nc.tensor.matmul(
                                    o,
                                    sb_U[:, kk,
                                         g * UNITS + mm * 128:
                                         g * UNITS + (mm + 1) * 128],
                                    hTb[:, kk * BLOC:(kk + 1) * BLOC],
                                    start=(kk == 0), stop=(kk == KU - 1),
                                )
                    tzr = wpool.tile([128, 2 * GW], f32, tag="tzr")
                    # z/r pre-activation adds (separate psum banks)
                    nc.vector.tensor_add(
                        tzr[:, 0:GW].rearrange("p (a b) -> p a b", b=BLOC),
                        pz[:].rearrange("p (a b) -> p a b", b=BLOC),
                        xinT[:, 0:KU, t * BLOC:(t + 1) * BLOC],
                    )
                    nc.vector.tensor_add(
                        tzr[:, GW:2 * GW].rearrange("p (a b) -> p a b", b=BLOC),
                        pr[:].rearrange("p (a b) -> p a b", b=BLOC),
                        xinT[:, KU:2 * KU, t * BLOC:(t + 1) * BLOC],
                    )
                    szr = wpool.tile([128, 2 * GW], f32, tag="szr")
                    nc.scalar.activation(
                        szr[:], tzr[:], mybir.ActivationFunctionType.Sigmoid)
                    zg = szr[:, 0:GW]
                    rg = szr[:, GW:2 * GW]

                    q0 = wpool.tile([128, GW], f32, tag="q0")
                    nc.vector.tensor_add(q0[:], ph[:], sb_b1h[:])
                    q1 = wpool.tile([128, GW], f32, tag="q1")
                    nc.vector.tensor_mul(q1[:], rg, q0[:])
                    q2 = wpool.tile([128, GW], f32, tag="q2")
                    nc.vector.tensor_add(
                        q2[:].rearrange("p (a b) -> p a b", b=BLOC),
                        q1[:].rearrange("p (a b) -> p a b", b=BLOC),
                        xinT[:, 2 * KU:3 * KU, t * BLOC:(t + 1) * BLOC],
                    )
                    hh = wpool.tile([128, GW], f32, tag="hh")
                    nc.scalar.activation(
                        hh[:], q2[:], mybir.ActivationFunctionType.Tanh)

                    d = wpool.tile([128, GW], f32, tag="d")
                    nc.vector.tensor_sub(d[:], hTf[:], hh[:])
                    e = wpool.tile([128, GW], f32, tag="e")
                    nc.vector.tensor_mul(e[:], zg, d[:])
                    hnew = wpool.tile([128, GW], f32, tag="hnew")
                    nc.vector.tensor_add(hnew[:], hh[:], e[:])
                    hTb = wpool.tile([128, GW], bf16, tag="hTbn")
                    nc.vector.tensor_copy(hTb[:], hnew[:])
                    hTf = hnew
                    nc.sync.dma_start(yT[:, t * GW:(t + 1) * GW], hnew[:])
    nc.finalize()
    return nc


_NC_CACHE = {}


def _get_nc(t_steps):
    if t_steps not in _NC_CACHE:
        _NC_CACHE[t_steps] = build_nc(t_steps)
    return _NC_CACHE[t_steps]


def kernel(x, hidden, E, W, U, b, _t_steps=T):
    x = np.asarray(x)
    hidden = np.asarray(hidden, dtype=np.float32)
    E = np.asarray(E, dtype=np.float32)
    W = np.asarray(W, dtype=np.float32)
    U = np.asarray(U, dtype=np.float32)
    b = np.asarray(b, dtype=np.float32)
    ts = _t_steps

    E_bf = np.ascontiguousarray(E.astype(BF))
    W_bf = np.ascontiguousarray(W.astype(BF))
    U_bf = np.ascontiguousarray(U.astype(BF))

    # bias0 col m = b0[m*128+p], plus b1 folded in for the z/r blocks
    bsum = b[0] + np.where(np.arange(3 * UNITS) < 2 * UNITS, b[1], 0.0)
    bias0 = np.ascontiguousarray(
        bsum.reshape(MU, 128).T.astype(np.float32))           # [128, MU]
    b1h = np.ascontiguousarray(
        np.repeat(b[1, 2 * UNITS:].reshape(KU, 128).T[:, :, None],
                  BLOC, axis=2).reshape(128, GW).astype(np.float32))

    in_maps = []
    for c in range(NCORES):
        xl = x[c * BLOC:(c + 1) * BLOC]                       # [8, T]
        xflat = xl.T[:ts].reshape(-1).astype(np.int32)        # r = t*8+b
        idx = np.ascontiguousarray(
            xflat.reshape((BLOC * ts) // 128, 128).T)         # [128, BT/128]
        hl = hidden[c * BLOC:(c + 1) * BLOC]                  # [8, 1024]
        hT0 = np.ascontiguousarray(
            hl.T.reshape(KU, 128, BLOC).transpose(1, 0, 2).reshape(128, GW)
            .astype(np.float32))
        in_maps.append({
            "E": E_bf, "W": W_bf, "U": U_bf, "idx": idx,
            "hT0": hT0, "bias0": bias0, "b1h": b1h,
        })

    nc = _get_nc(ts)
    res = bass_utils.run_bass_kernel_spmd(nc, in_maps, core_ids=list(range(NCORES)))
    LAST_RESULTS["res"] = res

    outputs = np.empty((B, ts, UNITS), np.float32)
    for c in range(NCORES):
        yT = res.results[c]["yT"]                             # [128, ts*GW]
        # col t*GW + kk*8 + b  ->  y[b, t, kk*128+p]
        arr = yT.reshape(128, ts, KU, BLOC).transpose(3, 1, 2, 0)
        outputs[c * BLOC:(c + 1) * BLOC] = arr.reshape(BLOC, ts, UNITS)
    state = np.ascontiguousarray(outputs[:, -1, :])
    return outputs, state


# revision 5
# speedup vs baseline: 1.0384x; 1.0384x over previous
"""GRU encoder (nn_Encoder) Trainium2 Bass kernel.

Model: emb = E[x]; xin = emb @ W + b[0]; then T=128 GRU steps (reset_after):
    rec = h @ U + b[1]
    z = sigmoid(xin_z + rec_z); r = sigmoid(xin_r + rec_r)
    hh = tanh(xin_h + r * rec_h)
    h  = z*h + (1-z)*hh

Sharding: data-parallel over batch, 8 rows per core. All weights replicated
(bf16); state/gates kept transposed (units on partitions, batch on free dim)
so the recurrent matmul out^T = U^T @ h^T lands directly in gate layout.

Per-core layouts:
  hT   [128, 64]      col kk*8+b  = h[b, kk*128+p]          (kk = unit block)
  xinT [128, 24, BT]  [p, m, t*8+b] = xin[b, t, m*128+p]
  yT   [128, T*64]    col t*64+kk*8+b = h_t[b, kk*128+p]
"""

import numpy as np
import ml_dtypes

import concourse.bass as bass
import concourse.mybir as mybir
from concourse import bacc
from concourse.tile import TileContext
from concourse import bass_utils
from concourse.masks import make_identity

BF = ml_dtypes.bfloat16
VOCAB, EMB, UNITS, B, T = 32000, 256, 1024, 64, 128
NCORES = 8
BLOC = B // NCORES          # 8 batch rows per core
BT = BLOC * T               # 1024 gathered rows per core
KU = UNITS // 128           # 8 K tiles over hidden units
MU = 3 * UNITS // 128       # 24 M tiles over rec columns
GW = BLOC * KU              # 64: gate tile free width (8 unit-blocks x 8 batch)

f32 = mybir.dt.float32
bf16 = mybir.dt.bfloat16
i32 = mybir.dt.int32

LAST_RESULTS = {}           # test harness peeks here for exec_time_ns


def build_nc(t_steps=T, has_b=False):
    bt = BLOC * t_steps
    nch = min(512, bt)
    nc = bacc.Bacc("TRN2")
    Ein = nc.dram_tensor("E", [VOCAB, EMB], bf16, kind="ExternalInput")
    Win = nc.dram_tensor("W", [EMB, 3 * UNITS], bf16, kind="ExternalInput")
    Uin = nc.dram_tensor("U", [UNITS, 3 * UNITS], bf16, kind="ExternalInput")
    idx_in = nc.dram_tensor("idx", [128, bt // 128], i32, kind="ExternalInput")
    hT0_in = nc.dram_tensor("hT0", [128, GW], f32, kind="ExternalInput")
    bias0_in = nc.dram_tensor("bias0", [128, MU], f32, kind="ExternalInput")
    b1h_in = nc.dram_tensor("b1h", [128, GW], f32, kind="ExternalInput")
    yT = nc.dram_tensor("yT", [128, t_steps * GW], bf16, kind="ExternalOutput")

    with TileContext(nc) as tc:
        with (
            tc.tile_pool(name="const", bufs=1) as cpool,
            tc.tile_pool(name="work", bufs=2) as wpool,
        ):
            ident = cpool.tile([128, 128], bf16)
            make_identity(nc, ident[:])
            idx_sb = cpool.tile([128, bt // 128], i32)
            nc.sync.dma_start(idx_sb[:], idx_in[:])
            sb_W = cpool.tile([128, EMB // 128, 3 * UNITS], bf16)
            nc.sync.dma_start(
                sb_W[:], Win[:].rearrange("(kk p) m -> p kk m", p=128)
            )
            sb_U = cpool.tile([128, KU, 3 * UNITS], bf16)
            nc.sync.dma_start(
                sb_U[:], Uin[:].rearrange("(kk p) m -> p kk m", p=128)
            )
            sb_bias0 = cpool.tile([128, MU], f32)
            nc.sync.dma_start(sb_bias0[:], bias0_in[:])
            sb_b1h = cpool.tile([128, GW], f32)
            nc.sync.dma_start(sb_b1h[:], b1h_in[:])

            hTf = cpool.tile([128, GW], f32, tag="hTf0")
            nc.sync.dma_start(hTf[:], hT0_in[:])
            hTb = cpool.tile([128, GW], bf16, tag="hTb0")
            nc.vector.tensor_copy(hTb[:], hTf[:])
            stt = nc.vector.scalar_tensor_tensor

            # Phase 1: embedding gather + transpose -> embT [128, 2, BT] bf16
            embT = cpool.tile([128, EMB // 128, bt], bf16)
            with tc.tile_pool(name="ptr", bufs=2, space="PSUM") as ptr:
                for tt in range(bt // 128):
                    etile = wpool.tile([128, EMB], bf16, tag="etile")
                    nc.gpsimd.indirect_dma_start(
                        out=etile[:], out_offset=None, in_=Ein[:],
                        in_offset=bass.IndirectOffsetOnAxis(
                            ap=idx_sb[:, tt:tt + 1], axis=0),
                    )
                    for ee in range(EMB // 128):
                        pt = ptr.tile([128, 128], bf16)
                        nc.tensor.transpose(
                            out=pt[:], in_=etile[:, ee * 128:(ee + 1) * 128],
                            identity=ident[:])
                        nc.vector.tensor_copy(
                            out=embT[:, ee, tt * 128:(tt + 1) * 128], in_=pt[:])

            # Phase 2: input GEMM -> xinT [128, MU, BT] f32 (bias folded in)
            xinT = cpool.tile([128, MU, bt], f32)
            with tc.tile_pool(name="px", bufs=4, space="PSUM") as pxp:
                for mm in range(MU):
                    for nh in range(bt // nch):
                        px = pxp.tile([128, nch], f32)
                        for kk in range(EMB // 128):
                            nc.tensor.matmul(
                                px[:],
                                sb_W[:, kk, mm * 128:(mm + 1) * 128],
                                embT[:, kk, nh * nch:(nh + 1) * nch],
                                start=(kk == 0), stop=(kk == EMB // 128 - 1),
                            )
                        nc.scalar.activation(
                            out=xinT[:, mm, nh * nch:(nh + 1) * nch], in_=px[:],
                            func=mybir.ActivationFunctionType.Identity,
                            bias=sb_bias0[:, mm:mm + 1],
                            scale=(2.0 if (not has_b or True) and mm >= 2 * KU else 1.0),
                        )

            # Phase 3: recurrence
            with tc.tile_pool(name="pg", bufs=2, space="PSUM") as pgp:
                for t in range(t_steps):
                    pz = pgp.tile([128, GW], f32, tag="pz")
                    pr = pgp.tile([128, GW], f32, tag="pr")
                    ph = pgp.tile([128, GW], f32, tag="ph")
                    for g, pg in ((0, pz), (1, pr), (2, ph)):
                        for mm in range(KU):
                            o = pg[:, mm * BLOC:(mm + 1) * BLOC]
                            for kk in range(KU):
                                nc.tensor.matmul(
                                    o,
                                    sb_U[:, kk,
                                         g * UNITS + mm * 128:
                                         g * UNITS + (mm + 1) * 128],
                                    hTb[:, kk * BLOC:(kk + 1) * BLOC],
                                    start=(kk == 0), stop=(kk == KU - 1),
                                )
                    tzr = wpool.tile([128, 2 * GW], f32, tag="tzr")
                    # z/r pre-activation adds (separate psum banks)
                    nc.vector.tensor_add(
                        tzr[:, 0:GW].rearrange("p (a b) -> p a b", b=BLOC),
                        pz[:].rearrange("p (a b) -> p a b", b=BLOC),
                        xinT[:, 0:KU, t * BLOC:(t + 1) * BLOC],
                    )
                    nc.vector.tensor_add(
                        tzr[:, GW:2 * GW].rearrange("p (a b) -> p a b", b=BLOC),
                        pr[:].rearrange("p (a b) -> p a b", b=BLOC),
                        xinT[:, KU:2 * KU, t * BLOC:(t + 1) * BLOC],
                    )
                    # all-tanh: sigmoid(x) = (tanh(x/2)+1)/2, folded into the
                    # downstream algebra so both activations share one table
                    svr = wpool.tile([128, 2 * GW], f32, tag="svr")
                    nc.scalar.activation(
                        svr[:], tzr[:], mybir.ActivationFunctionType.Tanh,
                        scale=0.5)
                    ug = svr[:, 0:GW]
                    vg = svr[:, GW:2 * GW]

                    w = wpool.tile([128, GW], f32, tag="w")
                    if has_b:
                        q0 = wpool.tile([128, GW], f32, tag="q0")
                        nc.vector.tensor_add(q0[:], ph[:], sb_b1h[:])
                        stt(w[:], vg, 1.0, q0[:],
                            mybir.AluOpType.add, mybir.AluOpType.mult)
                    else:
                        stt(w[:], vg, 1.0, ph[:],
                            mybir.AluOpType.add, mybir.AluOpType.mult)
                    q2 = wpool.tile([128, GW], f32, tag="q2")
                    nc.vector.tensor_add(
                        q2[:].rearrange("p (a b) -> p a b", b=BLOC),
                        w[:].rearrange("p (a b) -> p a b", b=BLOC),
                        xinT[:, 2 * KU:3 * KU, t * BLOC:(t + 1) * BLOC],
                    )
                    hh = wpool.tile([128, GW], f32, tag="hh")
                    nc.scalar.activation(
                        hh[:], q2[:], mybir.ActivationFunctionType.Tanh,
                        scale=0.5)

                    # h' = (h+hh)/2 + u*(h-hh)/2, state carried in bf16
                    A = wpool.tile([128, GW], f32, tag="A")
                    nc.vector.tensor_add(A[:], hTb[:], hh[:])
                    Bm = wpool.tile([128, GW], f32, tag="Bm")
                    nc.vector.tensor_sub(Bm[:], hTb[:], hh[:])
                    Cm = wpool.tile([128, GW], f32, tag="Cm")
                    stt(Cm[:], ug, 0.5, Bm[:],
                        mybir.AluOpType.mult, mybir.AluOpType.mult)
                    hTb = wpool.tile([128, GW], bf16, tag="hTbn")
                    stt(hTb[:], A[:], 0.5, Cm[:],
                        mybir.AluOpType.mult, mybir.AluOpType.add)
                    nc.sync.dma_start(yT[:, t * GW:(t + 1) * GW], hTb[:])
    nc.finalize()
    return nc


_NC_CACHE = {}


def _get_nc(t_steps, has_b):
    key = (t_steps, has_b)
    if key not in _NC_CACHE:
        _NC_CACHE[key] = build_nc(t_steps, has_b)
    return _NC_CACHE[key]


def kernel(x, hidden, E, W, U, b, _t_steps=T):
    x = np.asarray(x)
    hidden = np.asarray(hidden, dtype=np.float32)
    E = np.asarray(E, dtype=np.float32)
    W = np.asarray(W, dtype=np.float32)
    U = np.asarray(U, dtype=np.float32)
    b = np.asarray(b, dtype=np.float32)
    ts = _t_steps

    E_bf = np.ascontiguousarray(E.astype(BF))
    W_bf = np.ascontiguousarray(W.astype(BF))
    U_bf = np.ascontiguousarray(U.astype(BF))

    has_b = bool(np.any(b))
    # bias0 col m: z/r blocks get b0+b1; h blocks get 2*b0 (the h-side input
    # is pre-scaled by 2 for the all-tanh gate algebra)
    bsum = np.where(np.arange(3 * UNITS) < 2 * UNITS, b[0] + b[1], 2.0 * b[0])
    bias0 = np.ascontiguousarray(
        bsum.reshape(MU, 128).T.astype(np.float32))           # [128, MU]
    b1h = np.ascontiguousarray(
        np.repeat(b[1, 2 * UNITS:].reshape(KU, 128).T[:, :, None],
                  BLOC, axis=2).reshape(128, GW).astype(np.float32))

    in_maps = []
    for c in range(NCORES):
        xl = x[c * BLOC:(c + 1) * BLOC]                       # [8, T]
        xflat = xl.T[:ts].reshape(-1).astype(np.int32)        # r = t*8+b
        idx = np.ascontiguousarray(
            xflat.reshape((BLOC * ts) // 128, 128).T)         # [128, BT/128]
        hl = hidden[c * BLOC:(c + 1) * BLOC]                  # [8, 1024]
        hT0 = np.ascontiguousarray(
            hl.T.reshape(KU, 128, BLOC).transpose(1, 0, 2).reshape(128, GW)
            .astype(np.float32))
        in_maps.append({
            "E": E_bf, "W": W_bf, "U": U_bf, "idx": idx,
            "hT0": hT0, "bias0": bias0, "b1h": b1h,
        })

    nc = _get_nc(ts, has_b)
    res = bass_utils.run_bass_kernel_spmd(nc, in_maps, core_ids=list(range(NCORES)))
    LAST_RESULTS["res"] = res

    outputs = np.empty((B, ts, UNITS), np.float32)
    for c in range(NCORES):
        yT = res.results[c]["yT"].astype(np.float32)          # [128, ts*GW]
        # col t*GW + kk*8 + b  ->  y[b, t, kk*128+p]
        arr = yT.reshape(128, ts, KU, BLOC).transpose(3, 1, 2, 0)
        outputs[c * BLOC:(c + 1) * BLOC] = arr.reshape(BLOC, ts, UNITS)
    state = np.ascontiguousarray(outputs[:, -1, :])
    return outputs, state


# revision 10
# speedup vs baseline: 1.0418x; 1.0033x over previous
"""GRU encoder (nn_Encoder) Trainium2 Bass kernel.

Model: emb = E[x]; xin = emb @ W + b[0]; then T=128 GRU steps (reset_after):
    rec = h @ U + b[1]
    z = sigmoid(xin_z + rec_z); r = sigmoid(xin_r + rec_r)
    hh = tanh(xin_h + r * rec_h)
    h  = z*h + (1-z)*hh

Sharding: data-parallel over batch, 8 rows per core. All weights replicated
(bf16); state/gates kept transposed (units on partitions, batch on free dim)
so the recurrent matmul out^T = U^T @ h^T lands directly in gate layout.

Per-core layouts:
  hT   [128, 64]      col kk*8+b  = h[b, kk*128+p]          (kk = unit block)
  xinT [128, 24, BT]  [p, m, t*8+b] = xin[b, t, m*128+p]
  yT   [128, T*64]    col t*64+kk*8+b = h_t[b, kk*128+p]
"""

import numpy as np
import ml_dtypes

import concourse.bass as bass
import concourse.mybir as mybir
from concourse import bacc
from concourse.tile import TileContext
from concourse import bass_utils
from concourse.masks import make_identity

BF = ml_dtypes.bfloat16
VOCAB, EMB, UNITS, B, T = 32000, 256, 1024, 64, 128
NCORES = 8
BLOC = B // NCORES          # 8 batch rows per core
BT = BLOC * T               # 1024 gathered rows per core
KU = UNITS // 128           # 8 K tiles over hidden units
MU = 3 * UNITS // 128       # 24 M tiles over rec columns
GW = BLOC * KU              # 64: gate tile free width (8 unit-blocks x 8 batch)

f32 = mybir.dt.float32
bf16 = mybir.dt.bfloat16
i32 = mybir.dt.int32

LAST_RESULTS = {}           # test harness peeks here for exec_time_ns


def build_nc(t_steps=T, has_b=False):
    bt = BLOC * t_steps
    nch = min(512, bt)
    nc = bacc.Bacc("TRN2")
    Ein = nc.dram_tensor("E", [VOCAB, EMB], bf16, kind="ExternalInput")
    Win = nc.dram_tensor("W", [EMB, 3 * UNITS], bf16, kind="ExternalInput")
    Uin = nc.dram_tensor("U", [UNITS, 3 * UNITS], bf16, kind="ExternalInput")
    idx_in = nc.dram_tensor("idx", [128, bt // 128], i32, kind="ExternalInput")
    hT0_in = nc.dram_tensor("hT0", [128, GW], f32, kind="ExternalInput")
    bias0_in = nc.dram_tensor("bias0", [128, MU], f32, kind="ExternalInput")
    b1h_in = nc.dram_tensor("b1h", [128, GW], f32, kind="ExternalInput")
    yT = nc.dram_tensor("yT", [128, t_steps * GW], bf16, kind="ExternalOutput")

    with TileContext(nc) as tc:
        with (
            tc.tile_pool(name="const", bufs=1) as cpool,
            tc.tile_pool(name="work", bufs=2) as wpool,
        ):
            ident = cpool.tile([128, 128], bf16)
            make_identity(nc, ident[:])
            idx_sb = cpool.tile([128, bt // 128], i32)
            nc.sync.dma_start(idx_sb[:], idx_in[:])
            sb_W = cpool.tile([128, EMB // 128, 3 * UNITS], bf16)
            nc.sync.dma_start(
                sb_W[:], Win[:].rearrange("(kk p) m -> p kk m", p=128)
            )
            sb_U = cpool.tile([128, KU, 3 * UNITS], bf16)
            nc.sync.dma_start(
                sb_U[:], Uin[:].rearrange("(kk p) m -> p kk m", p=128)
            )
            sb_bias0 = cpool.tile([128, MU], f32)
            nc.sync.dma_start(sb_bias0[:], bias0_in[:])
            sb_b1h = cpool.tile([128, GW], f32)
            nc.sync.dma_start(sb_b1h[:], b1h_in[:])

            hTf = cpool.tile([128, GW], f32, tag="hTf0")
            nc.sync.dma_start(hTf[:], hT0_in[:])
            hTb = cpool.tile([128, GW], bf16, tag="hTb0")
            nc.vector.tensor_copy(hTb[:], hTf[:])
            stt = nc.vector.scalar_tensor_tensor

            # Phase 1: embedding gather + transpose -> embT [128, 2, BT] bf16
            embT = cpool.tile([128, EMB // 128, bt], bf16)
            with tc.tile_pool(name="ptr", bufs=2, space="PSUM") as ptr:
                for tt in range(bt // 128):
                    etile = wpool.tile([128, EMB], bf16, tag="etile")
                    nc.gpsimd.indirect_dma_start(
                        out=etile[:], out_offset=None, in_=Ein[:],
                        in_offset=bass.IndirectOffsetOnAxis(
                            ap=idx_sb[:, tt:tt + 1], axis=0),
                    )
                    for ee in range(EMB // 128):
                        pt = ptr.tile([128, 128], bf16)
                        nc.tensor.transpose(
                            out=pt[:], in_=etile[:, ee * 128:(ee + 1) * 128],
                            identity=ident[:])
                        nc.vector.tensor_copy(
                            out=embT[:, ee, tt * 128:(tt + 1) * 128], in_=pt[:])

            # Phase 2: input GEMM -> xinT [128, MU, BT] f32 (bias folded in)
            xinT = cpool.tile([128, MU, bt], f32)
            with tc.tile_pool(name="px", bufs=4, space="PSUM") as pxp:
                for mm in range(MU):
                    for nh in range(bt // nch):
                        px = pxp.tile([128, nch], f32)
                        for kk in range(EMB // 128):
                            nc.tensor.matmul(
                                px[:],
                                sb_W[:, kk, mm * 128:(mm + 1) * 128],
                                embT[:, kk, nh * nch:(nh + 1) * nch],
                                start=(kk == 0), stop=(kk == EMB // 128 - 1),
                            )
                        nc.scalar.activation(
                            out=xinT[:, mm, nh * nch:(nh + 1) * nch], in_=px[:],
                            func=mybir.ActivationFunctionType.Identity,
                            bias=sb_bias0[:, mm:mm + 1],
                            scale=(2.0 if (not has_b or True) and mm >= 2 * KU else 1.0),
                        )

            # Phase 3: recurrence
            with tc.tile_pool(name="pg", bufs=2, space="PSUM") as pgp:
                for t in range(t_steps):
                    pz = pgp.tile([128, GW], f32, tag="pz", name=f"pz_{t}")
                    pr = pgp.tile([128, GW], f32, tag="pr", name=f"pr_{t}")
                    ph = pgp.tile([128, GW], f32, tag="ph", name=f"ph_{t}")
                    for g, pg in ((0, pz), (1, pr), (2, ph)):
                        for mm in range(KU):
                            o = pg[:, mm * BLOC:(mm + 1) * BLOC]
                            for kk in range(KU):
                                nc.tensor.matmul(
                                    o,
                                    sb_U[:, kk,
                                         g * UNITS + mm * 128:
                                         g * UNITS + (mm + 1) * 128],
                                    hTb[:, kk * BLOC:(kk + 1) * BLOC],
                                    start=(kk == 0), stop=(kk == KU - 1),
                                )
                    tzr = wpool.tile([128, 2 * GW], f32, tag="tzr",
                                     name=f"tzr_{t}")
                    # z/r pre-activation adds (separate psum banks)
                    nc.vector.tensor_add(
                        tzr[:, 0:GW].rearrange("p (a b) -> p a b", b=BLOC),
                        pz[:].rearrange("p (a b) -> p a b", b=BLOC),
                        xinT[:, 0:KU, t * BLOC:(t + 1) * BLOC],
                    )
                    nc.vector.tensor_add(
                        tzr[:, GW:2 * GW].rearrange("p (a b) -> p a b", b=BLOC),
                        pr[:].rearrange("p (a b) -> p a b", b=BLOC),
                        xinT[:, KU:2 * KU, t * BLOC:(t + 1) * BLOC],
                    )
                    # all-tanh: sigmoid(x) = (tanh(x/2)+1)/2, folded into the
                    # downstream algebra so both activations share one table
                    svr = wpool.tile([128, 2 * GW], f32, tag="svr",
                                     name=f"svr_{t}")
                    nc.scalar.activation(
                        svr[:], tzr[:], mybir.ActivationFunctionType.Tanh,
                        scale=0.5)
                    ug = svr[:, 0:GW]
                    vg = svr[:, GW:2 * GW]

                    w = wpool.tile([128, GW], f32, tag="w", name=f"w_{t}")
                    if has_b:
                        q0 = wpool.tile([128, GW], f32, tag="q0",
                                        name=f"q0_{t}")
                        nc.vector.tensor_add(q0[:], ph[:], sb_b1h[:])
                        stt(w[:], vg, 1.0, q0[:],
                            mybir.AluOpType.add, mybir.AluOpType.mult)
                    else:
                        stt(w[:], vg, 1.0, ph[:],
                            mybir.AluOpType.add, mybir.AluOpType.mult)
                    q2 = wpool.tile([128, GW], f32, tag="q2", name=f"q2_{t}")
                    nc.vector.tensor_add(
                        q2[:].rearrange("p (a b) -> p a b", b=BLOC),
                        w[:].rearrange("p (a b) -> p a b", b=BLOC),
                        xinT[:, 2 * KU:3 * KU, t * BLOC:(t + 1) * BLOC],
                    )
                    hh = wpool.tile([128, GW], f32, tag="hh", name=f"hh_{t}")
                    nc.scalar.activation(
                        hh[:], q2[:], mybir.ActivationFunctionType.Tanh,
                        scale=0.5)

                    # h' = (h+hh)/2 + u*(h-hh)/2, state carried in bf16.
                    # A is off the dependency spine (only hnew consumes it):
                    # compute it on the otherwise-idle GpSimd so the DVE
                    # chain stays B -> C -> hnew.
                    A = wpool.tile([128, GW], f32, tag="A", name=f"A_{t}")
                    nc.gpsimd.tensor_add(A[:], hTb[:], hh[:])
                    Bm = wpool.tile([128, GW], f32, tag="Bm", name=f"Bm_{t}")
                    nc.vector.tensor_sub(Bm[:], hTb[:], hh[:])
                    Cm = wpool.tile([128, GW], f32, tag="Cm", name=f"Cm_{t}")
                    stt(Cm[:], ug, 0.5, Bm[:],
                        mybir.AluOpType.mult, mybir.AluOpType.mult)
                    hTb = wpool.tile([128, GW], bf16, tag="hTbn",
                                     name=f"hTbn_{t}")
                    stt(hTb[:], A[:], 0.5, Cm[:],
                        mybir.AluOpType.mult, mybir.AluOpType.add)
                    nc.sync.dma_start(yT[:, t * GW:(t + 1) * GW], hTb[:])
    nc.finalize()
    return nc


_NC_CACHE = {}


def _get_nc(t_steps, has_b):
    key = (t_steps, has_b)
    if key not in _NC_CACHE:
        _NC_CACHE[key] = build_nc(t_steps, has_b)
    return _NC_CACHE[key]


def kernel(x, hidden, E, W, U, b, _t_steps=T):
    x = np.asarray(x)
    hidden = np.asarray(hidden, dtype=np.float32)
    E = np.asarray(E, dtype=np.float32)
    W = np.asarray(W, dtype=np.float32)
    U = np.asarray(U, dtype=np.float32)
    b = np.asarray(b, dtype=np.float32)
    ts = _t_steps

    E_bf = np.ascontiguousarray(E.astype(BF))
    W_bf = np.ascontiguousarray(W.astype(BF))
    U_bf = np.ascontiguousarray(U.astype(BF))

    has_b = bool(np.any(b))
    # bias0 col m: z/r blocks get b0+b1; h blocks get 2*b0 (the h-side input
    # is pre-scaled by 2 for the all-tanh gate algebra)
    bsum = np.where(np.arange(3 * UNITS) < 2 * UNITS, b[0] + b[1], 2.0 * b[0])
    bias0 = np.ascontiguousarray(
        bsum.reshape(MU, 128).T.astype(np.float32))           # [128, MU]
    b1h = np.ascontiguousarray(
        np.repeat(b[1, 2 * UNITS:].reshape(KU, 128).T[:, :, None],
                  BLOC, axis=2).reshape(128, GW).astype(np.float32))

    in_maps = []
    for c in range(NCORES):
        xl = x[c * BLOC:(c + 1) * BLOC]                       # [8, T]
        xflat = xl.T[:ts].reshape(-1).astype(np.int32)        # r = t*8+b
        idx = np.ascontiguousarray(
            xflat.reshape((BLOC * ts) // 128, 128).T)         # [128, BT/128]
        hl = hidden[c * BLOC:(c + 1) * BLOC]                  # [8, 1024]
        hT0 = np.ascontiguousarray(
            hl.T.reshape(KU, 128, BLOC).transpose(1, 0, 2).reshape(128, GW)
            .astype(np.float32))
        in_maps.append({
            "E": E_bf, "W": W_bf, "U": U_bf, "idx": idx,
            "hT0": hT0, "bias0": bias0, "b1h": b1h,
        })

    nc = _get_nc(ts, has_b)
    res = bass_utils.run_bass_kernel_spmd(nc, in_maps, core_ids=list(range(NCORES)))
    LAST_RESULTS["res"] = res

    outputs = np.empty((B, ts, UNITS), np.float32)
    for c in range(NCORES):
        yT = res.results[c]["yT"].astype(np.float32)          # [128, ts*GW]
        # col t*GW + kk*8 + b  ->  y[b, t, kk*128+p]
        arr = yT.reshape(128, ts, KU, BLOC).transpose(3, 1, 2, 0)
        outputs[c * BLOC:(c + 1) * BLOC] = arr.reshape(BLOC, ts, UNITS)
    state = np.ascontiguousarray(outputs[:, -1, :])
    return outputs, state
